# revision 29
# baseline (speedup 1.0000x reference)
"""Trainium2 Bass kernel for an 8-layer densely-connected MLP.

Math: the reference's dense past/future skip-connection structure is linear
in the per-layer silu outputs a_i, so it collapses (host-side, fp64) to

    a_0 = silu(x @ W0.T + b0)
    a_i = silu(sum_{m<i} a_m @ P[i][m].T + bh[i-1])      i = 1..7
    out = log_softmax(a_7 @ Wout.T + bout)

with 28 precomputed 64x64 matrices P[i][m].

Device layout: activations are feature-major ([64 feat, N batch] tiles).
x.T is pre-transposed and cast to fp16 on the host so the device only ever
does full-rate contiguous DMA loads.  Each megatile is 2048 batch rows,
processed as two 1024-row "chunks" living in partition halves 0:64 / 64:128;
the two chunks run as concurrent 2D-tiled matmuls on the PE array
(tile_position row/col groups), and one [128, 512] Silu activation op covers
both chunks at full lane utilization.  Raw logits are PE-transposed to
batch-major and log-softmax runs once at the end (single ACT table switch).
"""

import sys

sys.path.insert(0, "/opt/trn_rl_repo")

import numpy as np

from contextlib import ExitStack

from concourse import bass, mybir, tile
from concourse.bass_utils import run_bass_kernel_spmd

# Problem constants (hardcoded per harness contract)
B, IN, H, OUT, L = 65536, 784, 64, 10, 8
N_CORES = 8
B_CORE = B // N_CORES            # 8192
MEGA = 2048                      # batch rows per megatile
N_MEGA = B_CORE // MEGA          # 4
CHUNK = MEGA // 2                # 1024 rows per partition-chunk
NH = CHUNK // 512                # column halves per chunk (2)
KBLK = 7                         # 784 = 7 * 112
KP = IN // KBLK                  # 112
NPAIR = L * (L - 1) // 2         # 28 (i,m) weight blocks
NBLK_T = MEGA // 128             # 16 transpose blocks per megatile
ACOLS = N_MEGA * NBLK_T * OUT    # out_acc columns (640)

f16 = mybir.dt.float16
f32 = mybir.dt.float32
AF = mybir.ActivationFunctionType


# ----------------------------------------------------------------------------
# Host-side weight preprocessing
# ----------------------------------------------------------------------------

def _precompute_P(Wh, bh, Wp, Wf):
    """Collapse past/future dense structure into P[(i, m)] (fp64)."""
    Wh = Wh.astype(np.float64)
    Wp = Wp.astype(np.float64)
    Wf = Wf.astype(np.float64)
    nl = L
    Z = np.zeros((H, H))
    S = {}
    for k in range(nl):
        for i in range(nl):
            S[(k, i)] = sum((Wf[k * (nl - 1) + (j - 1)] for j in range(i + 1, nl)), start=Z)
    G = {(0, 0): np.eye(H)}
    for i in range(1, nl):
        G[(i, i)] = np.eye(H) + S[(i, i)] if i < nl - 1 else np.eye(H)
        for m in range(i):
            G[(i, m)] = sum((S[(k, i)] @ G[(k, m)] for k in range(m, i)), start=Z)
    P = {}
    for i in range(1, nl):
        C = {j: Wh[i - 1] @ Wp[j * (nl - 1) + (i - 1)] for j in range(i)}
        for m in range(i):
            P[(i, m)] = sum((C[j] @ G[(j, m)] for j in range(m, i)), start=Z)
    return P


PAIR_INDEX = {}
for _i in range(1, L):
    for _m in range(_i):
        PAIR_INDEX[(_i, _m)] = len(PAIR_INDEX)


def _pack_weights(W0, b0, Wh, bh, Wp, Wf, Wout, bout):
    P = _precompute_P(Wh, bh, Wp, Wf)
    # W0.T in K-blocks of 112: [112, 7, 64]
    w0t = np.ascontiguousarray(
        W0.astype(np.float64).T.reshape(KBLK, KP, H).transpose(1, 0, 2)
    ).astype(np.float16)
    # P[i][m].T duplicated into both partition halves: [28, 128, 64]
    wpd = np.zeros((NPAIR, 128, H), np.float16)
    for (i, m), k in PAIR_INDEX.items():
        pt = P[(i, m)].T.astype(np.float16)
        wpd[k, 0:H] = pt
        wpd[k, H:128] = pt
    woutt_d = np.zeros((128, OUT), np.float16)
    woutt_d[0:H] = Wout.T.astype(np.float16)
    woutt_d[H:128] = Wout.T.astype(np.float16)
    # per-layer biases duplicated into both halves: [128, 8]
    bias8 = np.zeros((128, L), np.float32)
    bias8[0:H, 0] = b0
    bias8[H:128, 0] = b0
    for i in range(1, L):
        bias8[0:H, i] = bh[i - 1]
        bias8[H:128, i] = bh[i - 1]
    # bout broadcast over the 16 transpose blocks: [128, 160]
    boutb = np.tile(bout.astype(np.float32), (128, NBLK_T))
    ident = np.eye(OUT, dtype=np.float32)
    return dict(
        w0t=w0t, wpd=wpd, woutt_d=woutt_d, bias8=bias8, boutb=boutb, ident=ident
    )


# ----------------------------------------------------------------------------
# Device program
# ----------------------------------------------------------------------------

def build_nc(n_mega=N_MEGA, silu_via_sigmoid=False):
    nc = bass.Bass()
    b_core = n_mega * MEGA
    acols = n_mega * NBLK_T * OUT

    xt_e = nc.dram_tensor("xt", [KBLK, KP, b_core], f16, kind="ExternalInput")
    w0t_e = nc.dram_tensor("w0t", [KP, KBLK, H], f16, kind="ExternalInput")
    wpd_e = nc.dram_tensor("wpd", [NPAIR, 128, H], f16, kind="ExternalInput")
    woutt_e = nc.dram_tensor("woutt_d", [128, OUT], f16, kind="ExternalInput")
    bias8_e = nc.dram_tensor("bias8", [128, L], f32, kind="ExternalInput")
    boutb_e = nc.dram_tensor("boutb", [128, OUT * NBLK_T], f32, kind="ExternalInput")
    ident_e = nc.dram_tensor("ident", [OUT, OUT], f32, kind="ExternalInput")
    o_e = nc.dram_tensor("o", [128, acols], f32, kind="ExternalOutput")

    with tile.TileContext(nc) as tc, ExitStack() as ctx:
        consts = ctx.enter_context(tc.tile_pool(name="consts", bufs=1))
        xpool = ctx.enter_context(tc.tile_pool(name="xpool", bufs=56))
        tpool = ctx.enter_context(tc.tile_pool(name="tpool", bufs=1))
        lpool = ctx.enter_context(tc.tile_pool(name="lpool", bufs=2))
        apool = ctx.enter_context(tc.tile_pool(name="apool", bufs=1))
        pp = ctx.enter_context(tc.tile_pool(name="pp", bufs=3, space="PSUM"))
        p2 = ctx.enter_context(tc.tile_pool(name="p2", bufs=2, space="PSUM"))

        # constants; order matters — the first megatile's x loads go first so
        # PE can start within ~5us, small consts follow
        w0t_s = consts.tile([KP, KBLK, H], f16)
        wpd_s = consts.tile([128, NPAIR, H], f16)
        woutt_s = consts.tile([128, OUT], f16)
        bias_s = consts.tile([128, L], f32)
        boutb_s = consts.tile([128, OUT * NBLK_T], f32)
        ident_s = consts.tile([OUT, OUT], f32)

        xts = {}

        def load_xts(mg):
            # per-j tiles so layer-0 matmuls start as soon as their K-block
            # lands; chunk A rides the SP HWDGE ring, chunk B the ACT ring
            lo = mg * MEGA
            for j in range(KBLK):
                for ck in range(2):
                    xc = xpool.tile(
                        [KP, CHUNK], f16, tag="xts", name=f"x{mg}{ck}{j}"
                    )
                    eng = nc.sync if ck == 0 else nc.scalar
                    lo2 = lo + ck * CHUNK
                    eng.dma_start(xc[:], xt_e[j, :, lo2 : lo2 + CHUNK])
                    xts[(mg, ck, j)] = xc

        nc.sync.dma_start(w0t_s[:], w0t_e[:])
        nc.sync.dma_start(bias_s[:], bias8_e[:])
        for _mg in range(min(2, n_mega)):
            load_xts(_mg)
        nc.sync.dma_start(wpd_s[:], wpd_e[:].rearrange("k p m -> p k m"))
        nc.sync.dma_start(woutt_s[:], woutt_e[:])
        nc.sync.dma_start(boutb_s[:], boutb_e[:])
        nc.sync.dma_start(ident_s[:], ident_e[:])

        # Prime ACT/DVE vector clocks on the const DMAs so later activation
        # instructions need only a single sync wait (walrus's activation
        # encoding rejects multi-sem waits: "Too many sync wait commands").
        prim_a = consts.tile([128, 1], f32)
        nc.scalar.copy(prim_a[:], bias_s[:, 0:1])
        prim_v = consts.tile([128, 1], f32)
        nc.vector.tensor_copy(prim_v[:], boutb_s[:, 0:1])

        out_acc = apool.tile([128, acols], f32)

        def emit_silu(dst, src, bias_ap):
            # dst = silu(src + bias) = (src + bias) * sigmoid(src + bias)
            if not silu_via_sigmoid:
                nc.scalar.activation(dst, src, AF.Silu, bias=bias_ap)
            else:  # CoreSim lacks Silu; mathematically identical path
                sg = tpool.tile(list(dst.shape), f32, tag="sg", name="sg", bufs=2)
                nc.scalar.activation(sg[:], src, AF.Sigmoid, bias=bias_ap)
                nc.vector.scalar_tensor_tensor(
                    out=dst, in0=src, scalar=bias_ap, in1=sg[:],
                    op0=mybir.AluOpType.add, op1=mybir.AluOpType.mult,
                )

        def emit_l0(mg, T):
            ps = pp.tile([128, CHUNK], f32, tag="pre", name=f"ps0_{mg}")
            for h in range(NH):
                cs = slice(h * 512, (h + 1) * 512)
                for j in range(KBLK):
                    first = j == 0
                    last = j == KBLK - 1
                    nc.tensor.matmul(
                        ps[0:H, cs], w0t_s[:, j, :], xts[(mg, 0, j)][:, cs],
                        start=first, stop=last, skip_group_check=True,
                    )
                    nc.tensor.matmul(
                        ps[H:128, cs], w0t_s[:, j, :], xts[(mg, 1, j)][:, cs],
                        start=first, stop=last, skip_group_check=True,
                    )
            emit_silu(T[0][:], ps[:], bias_s[:, 0:1])

        def emit_dense(mg, T, i):
            ps = pp.tile([128, CHUNK], f32, tag="pre", name=f"ps{i}_{mg}")
            for h in range(NH):
                cs = slice(h * 512, (h + 1) * 512)
                for m in range(i):
                    k = PAIR_INDEX[(i, m)]
                    first = m == 0
                    last = m == i - 1
                    nc.tensor.matmul(
                        ps[0:H, cs], wpd_s[0:H, k, :], T[m][0:H, cs],
                        start=first, stop=last, skip_group_check=True,
                    )
                    nc.tensor.matmul(
                        ps[H:128, cs], wpd_s[H:128, k, :], T[m][H:128, cs],
                        start=first, stop=last, skip_group_check=True,
                    )
            emit_silu(T[i][:], ps[:], bias_s[:, i : i + 1])

        def emit_logits(mg, T):
            lgsT = lpool.tile([OUT, MEGA], f32, tag="lgsT", name=f"lg{mg}")
            for ck in range(2):
                for h in range(NH):
                    cs = slice(h * 512, (h + 1) * 512)
                    plg = pp.tile([OUT, 512], f32, tag="pre", name=f"plg{mg}")
                    nc.tensor.matmul(
                        plg[:],
                        woutt_s[ck * H : ck * H + H, :],
                        T[L - 1][ck * H : ck * H + H, cs],
                        start=True, stop=True,
                    )
                    seg = (ck * NH + h) * 512
                    nc.vector.tensor_copy(lgsT[:, seg : seg + 512], plg[:])
            return lgsT

        def emit_transpose(mg, lgsT):
            pt = p2.tile([128, OUT * NBLK_T], f32, tag="pt", name=f"pt{mg}")
            for blk in range(NBLK_T):
                nc.tensor.matmul(
                    pt[:, blk * OUT : (blk + 1) * OUT],
                    lgsT[:, blk * 128 : (blk + 1) * 128],
                    ident_s[:],
                    is_transpose=True,
                    start=True, stop=True, skip_group_check=True,
                )
            aseg = mg * OUT * NBLK_T
            nc.vector.tensor_add(
                out_acc[:, aseg : aseg + OUT * NBLK_T], pt[:], boutb_s[:]
            )

        # Two megatiles in flight, layer-interleaved: megatile B's matmuls
        # cover megatile A's silu latency so PE never drains (HAM stays warm).
        # The next pair's layer-0 is emitted before this pair's transposes so
        # PE has dense work while DVE stages the logits for transposition.
        assert n_mega % 2 == 0 or n_mega == 1
        pairs = [[m] for m in range(n_mega)] if n_mega == 1 else [
            [2 * p, 2 * p + 1] for p in range(n_mega // 2)
        ]

        def alloc_T(mg):
            return [
                tpool.tile([128, CHUNK], f16, tag=f"T{m}_{mg % 2}", name=f"T{m}_{mg}")
                for m in range(L)
            ]

        Ts = {mg: alloc_T(mg) for mg in pairs[0]}
        for mg in pairs[0]:
            emit_l0(mg, Ts[mg])
        for pr, mgs in enumerate(pairs):
            nxt = pairs[pr + 1] if pr + 1 < len(pairs) else []
            for mg in nxt:
                if (mg, 0, 0) not in xts:
                    load_xts(mg)
            for i in range(1, L):
                for mg in mgs:
                    emit_dense(mg, Ts[mg], i)
            lgs = {mg: emit_logits(mg, Ts[mg]) for mg in mgs}
            for mg in nxt:
                Ts[mg] = alloc_T(mg)
                emit_l0(mg, Ts[mg])
            for mg in mgs:
                emit_transpose(mg, lgs[mg])

        # ---- deferred log-softmax over groups of 10, batch-major ----
        ngrp = acols // OUT
        ex = apool.tile([128, acols], f32)
        sm = apool.tile([128, ngrp], f32)
        lsm = apool.tile([128, ngrp], f32)
        od = apool.tile([128, acols], f32)
        nc.scalar.activation(ex[:], out_acc[:], AF.Exp)
        nc.vector.reduce_sum(
            out=sm[:], in_=ex[:].rearrange("p (g c) -> p g c", c=OUT),
            axis=mybir.AxisListType.X,
        )
        nc.scalar.activation(lsm[:], sm[:], AF.Ln)
        for c in range(OUT):
            nc.vector.tensor_sub(
                od[:].rearrange("p (g c) -> p g c", c=OUT)[:, :, c],
                out_acc[:].rearrange("p (g c) -> p g c", c=OUT)[:, :, c],
                lsm[:],
            )
        nc.sync.dma_start(o_e[:], od[:])

    _split_multi_waits(nc)
    return nc


def _split_multi_waits(nc):
    """walrus's activation encoding admits one sync-wait; hoist extras onto
    preceding same-engine NoOps (sequentially equivalent)."""
    for blk in nc.m.functions[0].blocks:
        idx = 0
        while idx < len(blk.instructions):
            inst = blk.instructions[idx]
            si = inst.sync_info
            splittable = isinstance(
                inst,
                (
                    mybir.InstActivation,
                    mybir.InstTensorCopy,
                    mybir.InstTensorTensor,
                    mybir.InstTensorReduce,
                    mybir.InstMatmult,
                    mybir.InstLdweights,
                    mybir.InstDMACopy,
                    mybir.InstMemset,
                    mybir.InstDrain,
                    mybir.InstStreamTranspose,
                ),
            )
            if splittable and si is not None and len(si.on_wait) > 1:
                extras = list(si.on_wait[:-1])
                si.on_wait = [si.on_wait[-1]]
                for w in reversed(extras):
                    nop = mybir.InstNoOp(
                        name=nc.get_next_instruction_name(), ins=[], outs=[]
                    )
                    nop.engine = inst.engine
                    nop.sync_info = mybir.SyncInfo(on_wait=[w], on_update=[])
                    nc.register_instruction(nop)
                    blk.instructions.insert(idx, nop)
                    idx += 1
            idx += 1


# ----------------------------------------------------------------------------
# Host wrapper
# ----------------------------------------------------------------------------

_CACHE = {}


def _get_nc(n_mega=N_MEGA):
    if n_mega not in _CACHE:
        _CACHE[n_mega] = build_nc(n_mega)
    return _CACHE[n_mega]


def prepare_inputs(x, W0, b0, Wh, bh, Wp, Wf, Wout, bout):
    consts = _pack_weights(W0, b0, Wh, bh, Wp, Wf, Wout, bout)
    # x.T as [7, 112, B] fp16, per-core slices
    xt16 = np.ascontiguousarray(x.T.astype(np.float16)).reshape(KBLK, KP, B)
    in_maps = []
    for c in range(N_CORES):
        m = dict(consts)
        m["xt"] = np.ascontiguousarray(xt16[:, :, c * B_CORE : (c + 1) * B_CORE])
        in_maps.append(m)
    return in_maps


def _unpermute(o_core):
    # o_core [128, 640] -> [8192, 10]; batch_local = mg*2048 + blk*128 + p
    return (
        o_core.reshape(128, N_MEGA, NBLK_T, OUT)
        .transpose(1, 2, 0, 3)
        .reshape(B_CORE, OUT)
    )


def run(inputs, trace=False, **kw):
    in_maps = prepare_inputs(**inputs)
    nc = _get_nc(N_MEGA)
    res = run_bass_kernel_spmd(nc, in_maps, list(range(N_CORES)), trace=trace, **kw)
    out = np.empty((B, OUT), np.float32)
    for c in range(N_CORES):
        out[c * B_CORE : (c + 1) * B_CORE] = _unpermute(res.results[c]["o"])
    return out, res


def kernel(**inputs):
    out, _ = run(inputs, trace=False)
    return out
